# revision 31
# baseline (speedup 1.0000x reference)
"""DTAModel (drug-target affinity) Trainium2 kernel — 8-core SPMD, single launch.

Sharding: node shards of 4096 (GCN path, dst-sharded edge aggregation with
AllGather of h between layers) + pair shards of 64 (protein CNN + regressor).
All float compute on device; host only builds index/coefficient tables.

v2: L1 aggregates padded-x directly (no xw1 stage), host-precomputed
coefficient matrices (S) and pooling one-hots, protein groups interleaved into
the GCN tile loop, fp32r single-pass matmuls for the conv/regressor path.
"""
import os
import sys
import types

import numpy as np

N_NODES = 32768
N_EDGES = 131072
BATCH = 512
SEQ_LEN = 1000
VOCAB = 26
EMB = 128
NCORES = 8
NSH = N_NODES // NCORES      # 4096 nodes per core
BSH = BATCH // NCORES        # 64 pairs per core
NT = NSH // 128              # 32 node tiles per core

F32 = np.float32
LAST_RES = None


def _install_ntff_shim():
    if "antenv.axon_hooks" in sys.modules:
        return
    mod = types.ModuleType("antenv.axon_hooks")
    holder = {"h": None}
    mod.set_axon_ntff_profile_hook = lambda h: holder.__setitem__("h", h)
    mod.get_axon_ntff_profile_hook = lambda: holder["h"]
    sys.modules["antenv.axon_hooks"] = mod
    try:
        from trn_agent_boot.trn_boot import _ntff_profile_via_ctypes
        so = "/opt/axon/libaxon_pjrt.so"
        if os.path.exists(so):
            mod.set_axon_ntff_profile_hook(_ntff_profile_via_ctypes(so))
    except Exception:
        pass


def _prep_edges(edge_index):
    """Edge tables: per-core dst-sorted 128-edge blocks (real edges only) +
    dense S matrices + per-node self-loop diag coefficient matrices."""
    src = np.asarray(edge_index[0], np.int64)
    dst = np.asarray(edge_index[1], np.int64)
    deg = (1.0 + np.bincount(dst, minlength=N_NODES)).astype(F32)
    dis = (1.0 / np.sqrt(deg)).astype(F32)
    coefs = (dis[src] * dis[dst]).astype(F32)
    order = np.argsort(dst, kind="stable")
    s_s, d_s, c_s = src[order], dst[order], coefs[order]

    tile_of = d_s // 128
    counts = np.bincount(tile_of, minlength=N_NODES // 128)
    bpt = int(np.ceil(counts.max() / 128))
    nblk = NT * bpt

    idx = np.zeros((NCORES, nblk * 128), np.int64)
    dstl = np.zeros((NCORES, nblk * 128), np.int64)
    coef = np.zeros((NCORES, nblk * 128), F32)
    tstart = np.concatenate([[0], np.cumsum(counts)])
    for gt in range(N_NODES // 128):
        c, t = divmod(gt, NT)
        lo, hi = tstart[gt], tstart[gt + 1]
        n = hi - lo
        base = t * bpt * 128
        idx[c, base:base + n] = s_s[lo:hi]
        dstl[c, base:base + n] = d_s[lo:hi] - gt * 128
        coef[c, base:base + n] = c_s[lo:hi]

    # dense S: S_all[c][e, 128*b + j] = coef * (dstl == j)
    S_all = np.zeros((NCORES, nblk, 128, 128), F32)
    bix = np.tile(np.arange(nblk)[:, None], (1, 128)).ravel()
    eix = np.tile(np.arange(128)[None, :], (nblk, 1)).ravel()
    for c in range(NCORES):
        S_all[c][bix, eix, dstl[c]] = coef[c]
    S_all = S_all.transpose(0, 2, 1, 3).reshape(NCORES, 128, nblk * 128).copy()

    # self-loop diag coefficients: D_all[c][p, t] = dis2[c*NSH + t*128 + p]
    dis2 = (dis * dis).astype(F32)
    D_all = np.ascontiguousarray(
        dis2.reshape(NCORES, NT, 128).transpose(0, 2, 1))

    nchunk = (nblk * 128 + 2047) // 2048
    pad = nchunk * 2048 - nblk * 128
    idx16 = np.concatenate(
        [idx, np.zeros((NCORES, pad), np.int64)], axis=1).astype(np.int16)
    wrapped = np.zeros((NCORES, 128, nchunk * 128), np.int16)
    for c in range(NCORES):
        w = idx16[c].reshape(nchunk, 128, 16)
        for ci in range(nchunk):
            blockw = w[ci].reshape(-1, 16).T
            wrapped[c, :, 128 * ci:128 * (ci + 1)] = np.tile(blockw, (8, 1))
    return bpt, nblk, nchunk, wrapped, S_all, D_all


def kernel(**inputs):
    global LAST_RES
    _install_ntff_shim()
    SKIP_PROT = bool(int(os.environ.get("DTA_SKIP_PROT", "0")))
    SKIP_GCN = bool(int(os.environ.get("DTA_SKIP_GCN", "0")))
    F32R = bool(int(os.environ.get("DTA_F32R", "1")))
    PREP = bool(int(os.environ.get("DTA_PREP", "0")))
    import concourse.bacc as bacc
    import concourse.tile as tile
    from concourse import hw_specs
    # Tile's static schedule uses this cost model; the default badly
    # underestimates SWDGE gather descriptor generation (~8 ns/desc measured),
    # which starves the PE stream of filler work during gather windows.
    hw_specs.TRN2Spec.SWDGE_NS_PER_DESCRIPTOR = 8.0
    from concourse import mybir
    from concourse.bass_utils import run_bass_kernel_spmd
    from concourse.library_config import mlp as mlp_lib

    g = lambda k: np.ascontiguousarray(np.asarray(inputs[k], F32))
    x = g("x")
    batch = np.asarray(inputs["batch"], np.int64)
    seq = np.asarray(inputs["protein_seq"], np.int64)
    W1 = g("W1")
    W2 = g("W2")
    W3 = g("W3")
    g1, bt1, g2, bt2, g3, bt3 = g("g1"), g("bt1"), g("g2"), g("bt2"), g("g3"), g("bt3")
    emb_w = g("emb")
    ck1, cb1, ck2, cb2, ck3, cb3 = g("ck1"), g("cb1"), g("ck2"), g("cb2"), g("ck3"), g("cb3")
    fw1, fb1, fw2, fb2, fw3, fb3 = g("fw1"), g("fb1"), g("fw2"), g("fb2"), g("fw3"), g("fb3")

    # ---------------- host tables ----------------
    bpt, nblk, nchunk, idx_w, S_host, D_host = _prep_edges(
        np.asarray(inputs["edge_index"]))

    cnt = np.bincount(batch, minlength=BATCH).astype(F32)
    cntinv = (1.0 / np.maximum(cnt, 1.0)).astype(F32)
    # pooling one-hots: P_all[c][n, (w*NT + t)*128 + j] = (batch[node] == 128w + j)
    P_host = np.zeros((NCORES, 128, 4 * NT * 128), F32)
    for c in range(NCORES):
        bl = batch[NSH * c:NSH * (c + 1)].reshape(NT, 128)
        for t in range(NT):
            w = bl[t] // 128
            j = bl[t] % 128
            P_host[c, np.arange(128), (w * NT + t) * 128 + j] = 1.0

    chvec = np.zeros((128, 6), F32)
    chvec[:64, 0], chvec[:64, 1] = g1, bt1
    chvec[:, 2], chvec[:, 3] = g2, bt2
    chvec[:, 4], chvec[:, 5] = g3, bt3

    pidx = np.zeros((128, 1), F32)
    pidx[:52, 0] = np.tile(np.arange(26, dtype=F32), 2)
    ident = np.eye(128, dtype=F32)
    xpad = np.zeros((N_NODES, 64), F32)
    xpad[:, :5] = x

    ids4 = seq.astype(F32).reshape(NCORES, BSH // 4, 4, SEQ_LEN).transpose(0, 2, 1, 3) \
        .reshape(NCORES, 4, (BSH // 4) * SEQ_LEN).copy()
    # 16-partition repack: ids16[c, si*4+q, j] = ids4[c, si, q*4000+j]
    ids16 = ids4.reshape(NCORES, 4, 4, 4000).reshape(NCORES, 16, 4000).copy()
    sel16 = np.zeros((16, 16 * VOCAB), F32)
    for s in range(16):
        sel16[s, VOCAB * s:VOCAB * (s + 1)] = 1.0
    ck1T = np.concatenate([ck1[:, :, t].T for t in range(4)], axis=1)
    ck2q = np.concatenate([np.tile(ck2[:, :, t].T, (4, 1)) for t in range(6)], axis=1)
    ck3d = np.concatenate([np.tile(ck3[:, :, t].T, (2, 1)) for t in range(8)], axis=1)
    fw1a = np.ascontiguousarray(fw1[:128])
    fw1b = np.ascontiguousarray(fw1[128:224])
    fb1c = np.ascontiguousarray(fb1.reshape(4, 128).T)
    fw2p = np.ascontiguousarray(fw2.reshape(4, 128, 256).transpose(1, 0, 2).reshape(128, 1024))
    fb2c = np.ascontiguousarray(fb2.reshape(2, 128).T)
    fw3p = np.ascontiguousarray(fw3.reshape(2, 128).T)

    # ---------------- device program ----------------
    dt = mybir.dt
    AF = mybir.ActivationFunctionType
    OP = mybir.AluOpType
    rg = [list(range(NCORES))]

    fr = dt.float32r if F32R else dt.float32

    nc = bacc.Bacc("TRN2", target_bir_lowering=False, debug=False, num_devices=NCORES)

    def din(name, shape, dty=dt.float32):
        return nc.dram_tensor(name, list(shape), dty, kind="ExternalInput")

    xpad_d = din("xpad", [N_NODES, 64], fr)
    xloc_d = din("xloc", [NSH, 64], fr)
    D_d = din("Dm", [128, NT])
    W1_d, W2_d, W3_d = din("W1", [5, 64], fr), din("W2", [64, 128], fr), din("W3", [128, 128], fr)
    chv_d = din("chv", [128, 6])
    ident_d = din("ident", [128, 128])
    pidx_d = din("pidx", [128, 1])
    idx_d = din("idxg", [128, nchunk * 128], dt.int16)
    S_d = din("Sm", [128, nblk * 128], fr)
    P_d = din("Pm", [128, 4 * NT * 128], fr)
    cntinv_d = din("cntinv", [64, 1])
    ids_d = din("ids16", [16, 4000], fr)
    emb_d = din("embw", [VOCAB, EMB], fr)
    sel_d = din("sel16", [16, 16 * VOCAB], fr)
    ck1_d, ck2_d, ck3_d = din("ck1T", [128, 128]), din("ck2q", [128, 384], fr), din("ck3d", [128, 768], fr)
    cb1_d, cb2_d, cb3_d = din("cb1", [32, 1]), din("cb2", [64, 1]), din("cb3", [96, 1])
    f1a_d, f1b_d = din("fw1a", [128, 512], fr), din("fw1b", [96, 512], fr)
    fb1_d = din("fb1c", [128, 4])
    f2_d, fb2_d = din("fw2p", [128, 1024], fr), din("fb2c", [128, 2])
    f3_d, fb3_d = din("fw3p", [128, 2], fr), din("fb3", [1, 1])
    out_d = nc.dram_tensor("out", [1, BSH], dt.float32, kind="ExternalOutput")

    with tile.TileContext(nc) as tc:
        with (
            tc.tile_pool(name="dram", bufs=1, space="DRAM") as dram,
            tc.tile_pool(name="cst", bufs=1) as cst,
            tc.tile_pool(name="big", bufs=1) as big,
            tc.tile_pool(name="wk", bufs=2) as wk,
            tc.tile_pool(name="wk1", bufs=1) as wk1,
            tc.tile_pool(name="wk3", bufs=1) as wk3,
            tc.tile_pool(name="gc", bufs=3) as gc,
            tc.tile_pool(name="sld", bufs=3) as sld,
            tc.tile_pool(name="gp", bufs=2, space="PSUM") as gp,
            tc.tile_pool(name="pp", bufs=4, space="PSUM") as pp,
        ):
            nc.gpsimd.load_library(mlp_lib)

            ag1_in = dram.tile([NSH, 64], fr)
            h1_full = dram.tile([N_NODES, 64], fr, addr_space="Shared")
            ag2_in = dram.tile([NSH, 128], fr)
            h2_full = dram.tile([N_NODES, 128], fr, addr_space="Shared")
            ar_ins = [dram.tile([128, 2], dt.float32, name=f"arin{i}") for i in range(3)]
            ar_outs = [dram.tile([128, 2], dt.float32, addr_space="Shared",
                                 name=f"arout{i}") for i in range(3)]
            arp_in = dram.tile([BATCH, 128], dt.float32)
            rsp_out = dram.tile([BSH, 128], dt.float32)

            def load(name, d, shape, dty=dt.float32):
                t = cst.tile(shape, dty, name=name)
                nc.sync.dma_start(t[:], d[:])
                return t

            ident_t = load("ident_t", ident_d, [128, 128])
            pidx_t = load("pidx_t", pidx_d, [128, 1])
            chv_t = load("chv_t", chv_d, [128, 6])
            W1_t = load("W1_t", W1_d, [5, 64], fr)
            W2_t = load("W2_t", W2_d, [64, 128], fr)
            W3_t = load("W3_t", W3_d, [128, 128], fr)
            idx_t = load("idx_t", idx_d, [128, nchunk * 128], dt.int16)
            dis2_t = load("dis2_t", D_d, [128, NT])
            xloc_t = cst.tile([128, NT * 64], fr, name="xloc_t")
            nc.sync.dma_start(
                xloc_t[:].rearrange("p (t j) -> p t j", j=64),
                xloc_d[:, :].rearrange("(t p) j -> p t j", p=128))
            cntinv_t = load("cntinv_t", cntinv_d, [64, 1])
            ids_t = load("ids_t", ids_d, [16, 4000], fr)
            emb_t = load("emb_t", emb_d, [VOCAB, EMB], fr)
            sel_t = load("sel_t", sel_d, [16, 16 * VOCAB], fr)
            ck1_t = load("ck1_t", ck1_d, [128, 128])
            ck2_t = load("ck2_t", ck2_d, [128, 384], fr)
            ck3_t = load("ck3_t", ck3_d, [128, 768], fr)
            cb1_t = load("cb1_t", cb1_d, [32, 1])
            cb2_t = load("cb2_t", cb2_d, [64, 1])
            cb3_t = load("cb3_t", cb3_d, [96, 1])
            f1a_t = load("f1a_t", f1a_d, [128, 512], fr)
            f1b_t = load("f1b_t", f1b_d, [96, 512], fr)
            fb1_t = load("fb1_t", fb1_d, [128, 4])
            f2_t = load("f2_t", f2_d, [128, 1024], fr)
            fb2_t = load("fb2_t", fb2_d, [128, 2])
            f3_t = load("f3_t", f3_d, [128, 2], fr)
            fb3_t = load("fb3_t", fb3_d, [1, 1])

            z_sb = big.tile([128, NSH], dt.float32)
            h3T = big.tile([128, NSH], dt.float32)
            hnat = big.tile([128, NSH], fr)
            protT = big.tile([96, BSH], fr)
            c1T = [big.tile([128, BSH], fr, name=f"c1T_{j}") for j in range(4)]
            c2T = [big.tile([128, BSH], fr, name=f"c2T_{j}") for j in range(2)]

            # ---------------- protein group ----------------
            def protein_group(gi):
                xq = []
                for si in range(4):
                    xq_s = wk1.tile([128, 1002], dt.float32, tag=f"xq{si}",
                                    name=f"xq{gi}_{si}")
                    nc.vector.memset(xq_s[:, 0:1], 0.0)
                    nc.vector.memset(xq_s[:, 1001:1002], 0.0)
                    xq.append(xq_s)
                for lc in range(2):
                    l0 = 500 * lc
                    for si in range(4):
                        off = SEQ_LEN * gi + l0
                        lane = si * 4 + off // 4000
                        col = off % 4000
                        pb = pp.tile([VOCAB, 500], dt.float32, space="PSUM", tag="pp",
                                     name=f"pb{gi}_{lc}_{si}")
                        nc.tensor.matmul(
                            pb[:], lhsT=sel_t[:, VOCAB * lane:VOCAB * (lane + 1)],
                            rhs=ids_t[:, col:col + 500],
                            start=True, stop=True)
                        oh = wk.tile([VOCAB, 500], fr, tag="oh",
                                     name=f"oh{gi}_{lc}_{si}")
                        nc.vector.tensor_scalar(oh[:], pb[:], pidx_t[:VOCAB, :], None,
                                                OP.is_equal)
                        pe = pp.tile([128, 500], dt.float32, space="PSUM", tag="pp",
                                     name=f"pe{gi}_{lc}_{si}")
                        nc.tensor.matmul(pe[:], lhsT=emb_t[:], rhs=oh[:],
                                         start=True, stop=True)
                        nc.scalar.activation(xq[si][:, 1 + l0:1 + l0 + 500], pe[:], AF.Copy)
                h1Q = wk.tile([128, 1003], fr, tag="h1Q", name=f"h1Q{gi}")
                nc.vector.memset(h1Q[:, 0:2].bitcast(dt.float32), 0.0)
                nc.vector.memset(h1Q[:, 1001:1003].bitcast(dt.float32), 0.0)
                for (l0, Lc) in ((0, 512), (512, 487)):
                    c1p = pp.tile([128, 512], dt.float32, space="PSUM", tag="pp",
                                  name=f"c1p{gi}_{l0}")
                    for tap in range(4):
                        for si in range(4):
                            nc.tensor.matmul(
                                c1p[32 * si:32 * (si + 1), :Lc],
                                lhsT=ck1_t[:, 32 * tap:32 * (tap + 1)],
                                rhs=xq[si][:, l0 + tap:l0 + tap + Lc],
                                start=(tap == 0), stop=(tap == 3),
                                tile_position=(0, 32 * si))
                    for si in range(4):
                        nc.scalar.activation(
                            h1Q[32 * si:32 * (si + 1), 2 + l0:2 + l0 + Lc],
                            c1p[32 * si:32 * (si + 1), :Lc], AF.Relu, bias=cb1_t[:])
                h2D = []
                for p in range(2):
                    h2p = wk.tile([128, 1005], fr, tag=f"h2D{p}",
                                  name=f"h2D{gi}_{p}")
                    nc.vector.memset(h2p[:, 0:3].bitcast(dt.float32), 0.0)
                    nc.vector.memset(h2p[:, 1001:1005].bitcast(dt.float32), 0.0)
                    h2D.append(h2p)
                for (l0, Lc) in ((0, 512), (512, 486)):
                    c2ps = []
                    for si in range(4):
                        c2p = pp.tile([64, 512], dt.float32, space="PSUM", tag="pp",
                                      name=f"c2p{gi}_{l0}_{si}")
                        c2ps.append(c2p)
                    for tap in range(6):
                        for si in range(4):
                            nc.tensor.matmul(
                                c2ps[si][:, :Lc],
                                lhsT=ck2_t[32 * si:32 * (si + 1),
                                              64 * tap:64 * (tap + 1)],
                                rhs=h1Q[32 * si:32 * (si + 1), l0 + tap:l0 + tap + Lc],
                                start=(tap == 0), stop=(tap == 5),
                                tile_position=(32 * si, 0))
                    for si in range(4):
                        nc.scalar.activation(
                            h2D[si // 2][64 * (si % 2):64 * (si % 2 + 1),
                                         3 + l0:3 + l0 + Lc],
                            c2ps[si][:, :Lc], AF.Relu, bias=cb2_t[:])
                for p in range(2):
                    mx = [wk.tile([96, 1], dt.float32, tag=f"mx{j}",
                                  name=f"mx{gi}_{p}_{j}") for j in range(2)]
                    tmp = [wk.tile([96, 1], dt.float32, tag=f"tm{j}",
                                   name=f"tm{gi}_{p}_{j}") for j in range(2)]
                    for (l0, Lc) in ((0, 512), (512, 486)):
                        Lreal = 512 if l0 == 0 else 485
                        c3ps = []
                        for j in range(2):
                            c3p = pp.tile([96, 512], dt.float32, space="PSUM", tag="pp",
                                          name=f"c3p{gi}_{p}_{l0}_{j}")
                            c3ps.append(c3p)
                        for tap in range(8):
                            for j in range(2):
                                nc.tensor.matmul(
                                    c3ps[j][:, :Lc],
                                    lhsT=ck3_t[64 * j:64 * (j + 1),
                                                  96 * tap:96 * (tap + 1)],
                                    rhs=h2D[p][64 * j:64 * (j + 1),
                                                  l0 + tap:l0 + tap + Lc],
                                    start=(tap == 0), stop=(tap == 7),
                                    tile_position=(64 * j, 0))
                        for j in range(2):
                            dst = mx[j] if l0 == 0 else tmp[j]
                            nc.vector.tensor_reduce(dst[:], c3ps[j][:, :Lreal],
                                                    axis=mybir.AxisListType.X, op=OP.max)
                            if l0 != 0:
                                nc.vector.tensor_tensor(mx[j][:], mx[j][:], tmp[j][:],
                                                        OP.max)
                    for j in range(2):
                        s_idx = 4 * gi + 2 * p + j
                        nc.scalar.activation(protT[:, s_idx:s_idx + 1], mx[j][:],
                                             AF.Relu, bias=cb3_t[:])

            pending = [] if SKIP_PROT else list(range(16))
            slot = [0]

            def filler(period=8):
                slot[0] += 1
                if pending and slot[0] % period == 0:
                    protein_group(pending.pop(0))

            # ---------------- GCN layer ----------------
            gsem = nc.alloc_semaphore("gsem")

            def gcn_layer(L, fg, fin, fout, src_dram, Wt, loc, lw):
                """fg: gathered row width; fin: contraction width of Wt.
                loc/lw: SBUF tile + per-tile stride holding the LOCAL input rows
                (natural layout) for the self-loop diag term."""
                zs = cst.tile([128, NT], dt.float32, name=f"zs{L}")
                zq = cst.tile([128, NT], dt.float32, name=f"zq{L}")
                sq_scr = wk.tile([128, 128], dt.float32, tag="sqs", name=f"sqs{L}")
                dsem = nc.alloc_semaphore(f"dsem{L}")
                chs = {}

                def emit_prep(ci):
                    g = gc.tile([128, 16, fg], fr, tag="gch", name=f"g{L}_{ci}")
                    chs[ci] = g
                    if PREP:
                        nc.gpsimd.dma_gather(
                            g[:], src_dram[:],
                            idx_t[:, 128 * ci:128 * (ci + 1)], 2048, 2048, fg,
                            single_packet=False, prepare_only=True, sem=dsem)
                        nc.gpsimd.trigger_dma(count=None)
                    else:
                        nc.gpsimd.dma_gather(
                            g[:], src_dram[:],
                            idx_t[:, 128 * ci:128 * (ci + 1)], 2048, 2048, fg,
                            single_packet=False)

                PRE = 2
                for ci in range(min(PRE, nchunk)):
                    emit_prep(ci)
                Gt = None
                for t in range(NT):
                    St = sld.tile([128, bpt * 128], fr, tag="Sld",
                                  name=f"S{L}_{t}")
                    nc.sync.dma_start(St[:], S_d[:, t * bpt * 128:(t + 1) * bpt * 128])
                    aggT = gp.tile([128, 128], dt.float32, space="PSUM", tag="aggp",
                                   name=f"agg{L}_{t}")
                    Dt = sld.tile([128, 128], fr, tag="Dt", name=f"D{L}_{t}")
                    nc.vector.tensor_scalar(Dt[:], ident_t[:], dis2_t[:, t:t + 1],
                                            None, OP.mult)
                    nc.tensor.matmul(aggT[:fin, :],
                                     lhsT=loc[:, lw * t:lw * t + fin],
                                     rhs=Dt[:],
                                     start=True, stop=False)
                    for k in range(bpt):
                        b = t * bpt + k
                        ci, bb = divmod(b, 16)
                        if bb == 0:
                            Gt = chs.pop(ci)
                            if ci + PRE < nchunk:
                                emit_prep(ci + PRE)
                        nc.tensor.matmul(aggT[:fin, :], lhsT=Gt[:, bb, :fin],
                                         rhs=St[:, 128 * k:128 * (k + 1)],
                                         start=False, stop=(k == bpt - 1))
                    aggS = wk.tile([fin, 128], fr, tag="aggS",
                                   name=f"aggS{L}_{t}")
                    nc.vector.tensor_copy(aggS[:], aggT[:fin, :])
                    zT = gp.tile([128, 128], dt.float32, space="PSUM", tag="zp",
                                 name=f"z{L}_{t}")
                    nc.tensor.matmul(zT[:fout, :], lhsT=Wt[:fin, :fout], rhs=aggS[:],
                                     start=True, stop=True)
                    nc.scalar.activation(z_sb[:fout, 128 * t:128 * (t + 1)], zT[:fout, :],
                                         AF.Copy, accum_out=zs[:fout, t:t + 1])
                    nc.scalar.activation(sq_scr[:fout, :], zT[:fout, :], AF.Square,
                                         accum_out=zq[:fout, t:t + 1])
                    filler(10)
                ssum = wk.tile([128, 2], dt.float32, tag="ssum", name=f"ssum{L}")
                nc.vector.memset(ssum[:], 0.0)
                nc.vector.tensor_reduce(ssum[:fout, 0:1], zs[:fout, :],
                                        axis=mybir.AxisListType.X, op=OP.add)
                nc.vector.tensor_reduce(ssum[:fout, 1:2], zq[:fout, :],
                                        axis=mybir.AxisListType.X, op=OP.add)
                nc.sync.dma_start(ar_ins[L][:], ssum[:])
                nc.gpsimd.collective_compute(
                    "AllReduce", OP.add, replica_groups=rg,
                    ins=[ar_ins[L].opt()], outs=[ar_outs[L].opt()])
                stg = wk.tile([128, 2], dt.float32, tag="stg", name=f"stg{L}")
                nc.sync.dma_start(stg[:], ar_outs[L][:])
                vg = chv_t[:fout, 2 * L:2 * L + 1]
                vbt = chv_t[:fout, 2 * L + 1:2 * L + 2]
                mean = wk.tile([128, 1], dt.float32, tag="bnv0", name=f"mean{L}")
                ex2 = wk.tile([128, 1], dt.float32, tag="bnv1", name=f"ex2{L}")
                var = wk.tile([128, 1], dt.float32, tag="bnv2", name=f"var{L}")
                sd = wk.tile([128, 1], dt.float32, tag="bnv3", name=f"sd{L}")
                s_ch = wk.tile([128, 1], dt.float32, tag="bnv4", name=f"sch{L}")
                b_ch = wk.tile([128, 1], dt.float32, tag="bnv5", name=f"bch{L}")
                t1 = wk.tile([128, 1], dt.float32, tag="bnv6", name=f"t1{L}")
                nc.vector.tensor_scalar(mean[:fout], stg[:fout, 0:1], 1.0 / N_NODES,
                                        None, OP.mult)
                nc.vector.tensor_scalar(ex2[:fout], stg[:fout, 1:2], 1.0 / N_NODES,
                                        None, OP.mult)
                nc.vector.tensor_tensor(var[:fout], mean[:fout], mean[:fout], OP.mult)
                nc.vector.tensor_tensor(var[:fout], ex2[:fout], var[:fout], OP.subtract)
                nc.vector.tensor_scalar(var[:fout], var[:fout], 1e-5, None, OP.add)
                nc.scalar.activation(sd[:fout], var[:fout], AF.Sqrt)
                nc.vector.reciprocal(s_ch[:fout], sd[:fout])
                nc.vector.tensor_tensor(s_ch[:fout], s_ch[:fout], vg, OP.mult)
                nc.vector.tensor_tensor(t1[:fout], mean[:fout], s_ch[:fout], OP.mult)
                nc.vector.tensor_tensor(b_ch[:fout], vbt, t1[:fout], OP.subtract)
                return s_ch, b_ch

            def apply_bn(L, fout, s_ch, b_ch, to_h3T):
                for t in range(NT):
                    if to_h3T:
                        nc.scalar.activation(
                            h3T[:fout, 128 * t:128 * (t + 1)],
                            z_sb[:fout, 128 * t:128 * (t + 1)],
                            AF.Relu, bias=b_ch[:fout], scale=s_ch[:fout])
                    else:
                        hT = wk.tile([128, 128], dt.float32, tag="hT", name=f"hT{L}_{t}")
                        nc.scalar.activation(
                            hT[:fout, :], z_sb[:fout, 128 * t:128 * (t + 1)],
                            AF.Relu, bias=b_ch[:fout], scale=s_ch[:fout])
                        tp = gp.tile([128, 128], dt.float32, space="PSUM", tag="zp",
                                     name=f"tp{L}_{t}")
                        nc.tensor.transpose(tp[:, :fout], hT[:fout, :],
                                            ident_t[:fout, :fout])
                        nc.vector.tensor_copy(hnat[:, fout * t:fout * (t + 1)],
                                              tp[:, :fout])

            def _emit_regressor(drugT):
                for jc in range(4):
                    f1ps = pp.tile([128, 64], dt.float32, space="PSUM", tag="pp",
                                   name=f"f1ps{jc}")
                    nc.tensor.matmul(f1ps[:], lhsT=f1a_t[:, 128 * jc:128 * (jc + 1)],
                                     rhs=drugT[:], start=True, stop=False)
                    nc.tensor.matmul(f1ps[:], lhsT=f1b_t[:, 128 * jc:128 * (jc + 1)],
                                     rhs=protT[:], start=False, stop=True)
                    nc.scalar.activation(c1T[jc][:, :], f1ps[:], AF.Relu,
                                         bias=fb1_t[:, jc:jc + 1])
                for jc in range(2):
                    f2ps = pp.tile([128, 64], dt.float32, space="PSUM", tag="pp",
                                   name=f"f2ps{jc}")
                    for ic in range(4):
                        nc.tensor.matmul(
                            f2ps[:],
                            lhsT=f2_t[:, 256 * ic + 128 * jc:256 * ic + 128 * jc + 128],
                            rhs=c1T[ic][:, :], start=(ic == 0), stop=(ic == 3))
                    nc.scalar.activation(c2T[jc][:, :], f2ps[:], AF.Relu,
                                         bias=fb2_t[:, jc:jc + 1])
                f3ps = pp.tile([1, 64], dt.float32, space="PSUM", tag="pp", name="f3ps0")
                for ic in range(2):
                    nc.tensor.matmul(f3ps[:], lhsT=f3_t[:, ic:ic + 1],
                                     rhs=c2T[ic][:, :],
                                     start=(ic == 0), stop=(ic == 1))
                outs = wk.tile([1, 64], dt.float32, tag="outs", name="outs0")
                nc.vector.tensor_scalar(outs[:], f3ps[:], fb3_t[:1, 0:1], None, OP.add)
                nc.sync.dma_start(out_d[:], outs[:])

            # ================= emission =================
            if SKIP_GCN:
                for gi in list(pending):
                    protein_group(gi)
                pending.clear()
                drugT0 = wk.tile([128, 64], fr, tag="drugT", name="drugT0")
                nc.vector.memset(drugT0[:].bitcast(dt.float32), 0.0)
                _emit_regressor(drugT0)
            else:
                if pending:
                    protein_group(pending.pop(0))
                s1, bb1 = gcn_layer(0, 64, 5, 64, xpad_d, W1_t, xloc_t, 64)
                apply_bn(0, 64, s1, bb1, False)
                view1 = ag1_in[:, :].rearrange("(t p) j -> p t j", p=128)
                nc.sync.dma_start(view1,
                                  hnat[:, :NT * 64].rearrange("p (t j) -> p t j", j=64))
                nc.gpsimd.collective_compute("AllGather", OP.bypass, replica_groups=rg,
                                             ins=[ag1_in.opt()], outs=[h1_full.opt()])
                filler(1)
                filler(1)
                filler(1)

                s2c, bb2 = gcn_layer(1, 64, 64, 128, h1_full, W2_t, hnat, 64)
                apply_bn(1, 128, s2c, bb2, False)
                view2 = ag2_in[:, :].rearrange("(t p) j -> p t j", p=128)
                nc.sync.dma_start(view2,
                                  hnat[:, :NT * 128].rearrange("p (t j) -> p t j", j=128))
                nc.gpsimd.collective_compute("AllGather", OP.bypass, replica_groups=rg,
                                             ins=[ag2_in.opt()], outs=[h2_full.opt()])
                filler(1)
                filler(1)
                filler(1)

                s3c, bb3 = gcn_layer(2, 128, 128, 128, h2_full, W3_t, hnat, 128)
                apply_bn(2, 128, s3c, bb3, True)

                for t in range(NT):
                    tpp = gp.tile([128, 128], dt.float32, space="PSUM", tag="zp",
                                  name=f"tpp_{t}")
                    nc.tensor.transpose(tpp[:], h3T[:, 128 * t:128 * (t + 1)], ident_t[:])
                    nc.vector.tensor_copy(hnat[:, 128 * t:128 * (t + 1)], tpp[:])
                    filler(4)
                for w in range(4):
                    poolw = gp.tile([128, 128], dt.float32, space="PSUM", tag="aggp",
                                    name=f"poolps{w}")
                    Pw = wk3.tile([128, NT * 128], fr, tag="p1h", name=f"p1_{w}")
                    nc.sync.dma_start(
                        Pw[:], P_d[:, w * NT * 128:(w + 1) * NT * 128])
                    for t in range(NT):
                        nc.tensor.matmul(
                            poolw[:], lhsT=Pw[:, 128 * t:128 * (t + 1)],
                            rhs=hnat[:, 128 * t:128 * (t + 1)],
                            start=(t == 0), stop=(t == NT - 1))
                    parts = wk.tile([128, 128], dt.float32, tag="parts", name=f"parts{w}")
                    nc.vector.tensor_copy(parts[:], poolw[:])
                    nc.sync.dma_start(arp_in[128 * w:128 * (w + 1), :], parts[:])
                    filler(2)
                nc.gpsimd.collective_compute("ReduceScatter", OP.add, replica_groups=rg,
                                             ins=[arp_in.opt()], outs=[rsp_out.opt()])
                drugsum = wk.tile([64, 128], dt.float32, tag="drugsum", name="drugsum0")
                nc.sync.dma_start(drugsum[:], rsp_out[:])
                drug = wk.tile([64, 128], dt.float32, tag="drug", name="drug0")
                nc.vector.tensor_scalar(drug[:], drugsum[:], cntinv_t[:], None, OP.mult)
                tpd = gp.tile([128, 128], dt.float32, space="PSUM", tag="zp", name="tpd0")
                nc.tensor.transpose(tpd[:, :64], drug[:], ident_t[:64, :64])
                drugT = wk.tile([128, 64], fr, tag="drugT", name="drugT0")
                nc.vector.tensor_copy(drugT[:], tpd[:, :64])

                while pending:
                    protein_group(pending.pop(0))
                _emit_regressor(drugT)

    nc.compile()

    in_maps = []
    for c in range(NCORES):
        in_maps.append({
            "xpad": xpad, "xloc": xpad[NSH * c:NSH * (c + 1)],
            "W1": W1, "W2": W2, "W3": W3, "chv": chvec,
            "ident": ident, "pidx": pidx,
            "idxg": idx_w[c], "Sm": S_host[c], "Pm": P_host[c], "Dm": D_host[c],
            "cntinv": cntinv[64 * c:64 * (c + 1)][:, None],
            "ids16": ids16[c], "embw": emb_w, "sel16": sel16,
            "ck1T": ck1T, "ck2q": ck2q, "ck3d": ck3d,
            "cb1": cb1[:, None], "cb2": cb2[:, None], "cb3": cb3[:, None],
            "fw1a": fw1a, "fw1b": fw1b, "fb1c": fb1c,
            "fw2p": fw2p, "fb2c": fb2c, "fw3p": fw3p,
            "fb3": np.array([[fb3[0]]], F32),
        })

    res = run_bass_kernel_spmd(nc, in_maps, core_ids=list(range(NCORES)))
    LAST_RES = res
    out = np.concatenate([res.results[c]["out"][0] for c in range(NCORES)])
    return out.astype(F32)



# revision 51
# speedup vs baseline: 1.2493x; 1.2493x over previous
"""DTAModel (drug-target affinity) Trainium2 kernel — 8-core SPMD, single launch.

Sharding: node shards of 4096 (GCN path, dst-sharded edge aggregation with
AllGather of h between layers) + pair shards of 64 (protein CNN + regressor).
All float compute on device; host only builds index/coefficient tables.

v2: L1 aggregates padded-x directly (no xw1 stage), host-precomputed
coefficient matrices (S) and pooling one-hots, protein groups interleaved into
the GCN tile loop, fp32r single-pass matmuls for the conv/regressor path.
"""
import os
import sys
import types

import numpy as np

N_NODES = 32768
N_EDGES = 131072
BATCH = 512
SEQ_LEN = 1000
VOCAB = 26
EMB = 128
NCORES = 8
NSH = N_NODES // NCORES      # 4096 nodes per core
BSH = BATCH // NCORES        # 64 pairs per core
NT = NSH // 128              # 32 node tiles per core

F32 = np.float32
LAST_RES = None


def _install_ntff_shim():
    if "antenv.axon_hooks" in sys.modules:
        return
    mod = types.ModuleType("antenv.axon_hooks")
    holder = {"h": None}
    mod.set_axon_ntff_profile_hook = lambda h: holder.__setitem__("h", h)
    mod.get_axon_ntff_profile_hook = lambda: holder["h"]
    sys.modules["antenv.axon_hooks"] = mod
    try:
        from trn_agent_boot.trn_boot import _ntff_profile_via_ctypes
        so = "/opt/axon/libaxon_pjrt.so"
        if os.path.exists(so):
            mod.set_axon_ntff_profile_hook(_ntff_profile_via_ctypes(so))
    except Exception:
        pass


def _prep_edges(edge_index):
    """Edge tables: per-core dst-sorted 128-edge blocks (real edges only) +
    dense S matrices + per-node self-loop diag coefficient matrices."""
    src = np.asarray(edge_index[0], np.int64)
    dst = np.asarray(edge_index[1], np.int64)
    deg = (1.0 + np.bincount(dst, minlength=N_NODES)).astype(F32)
    dis = (1.0 / np.sqrt(deg)).astype(F32)
    coefs = (dis[src] * dis[dst]).astype(F32)
    order = np.argsort(dst, kind="stable")
    s_s, d_s, c_s = src[order], dst[order], coefs[order]

    tile_of = d_s // 128
    counts = np.bincount(tile_of, minlength=N_NODES // 128)
    bpt = int(np.ceil(counts.max() / 128))
    nblk = NT * bpt

    idx = np.zeros((NCORES, nblk * 128), np.int64)
    dstl = np.zeros((NCORES, nblk * 128), np.int64)
    coef = np.zeros((NCORES, nblk * 128), F32)
    tstart = np.concatenate([[0], np.cumsum(counts)])
    for gt in range(N_NODES // 128):
        c, t = divmod(gt, NT)
        lo, hi = tstart[gt], tstart[gt + 1]
        n = hi - lo
        base = t * bpt * 128
        idx[c, base:base + n] = s_s[lo:hi]
        dstl[c, base:base + n] = d_s[lo:hi] - gt * 128
        coef[c, base:base + n] = c_s[lo:hi]

    # dense S: S_all[c][e, 128*b + j] = coef * (dstl == j)
    S_all = np.zeros((NCORES, nblk, 128, 128), F32)
    bix = np.tile(np.arange(nblk)[:, None], (1, 128)).ravel()
    eix = np.tile(np.arange(128)[None, :], (nblk, 1)).ravel()
    for c in range(NCORES):
        S_all[c][bix, eix, dstl[c]] = coef[c]
    S_all = S_all.transpose(0, 2, 1, 3).reshape(NCORES, 128, nblk * 128).copy()

    # self-loop diag: D_all[c][p, 128*t + j] = (p == j) * dis2[c*NSH + t*128 + p]
    dis2 = (dis * dis).astype(F32)
    D_all = np.zeros((NCORES, 128, NT * 128), F32)
    rr = np.arange(128)
    for c in range(NCORES):
        dl = dis2[NSH * c:NSH * (c + 1)].reshape(NT, 128)
        for t in range(NT):
            D_all[c, rr, t * 128 + rr] = dl[t]

    nchunk = (nblk * 128 + 2047) // 2048
    pad = nchunk * 2048 - nblk * 128
    idx16 = np.concatenate(
        [idx, np.zeros((NCORES, pad), np.int64)], axis=1).astype(np.int16)
    wrapped = np.zeros((NCORES, 128, nchunk * 128), np.int16)
    for c in range(NCORES):
        w = idx16[c].reshape(nchunk, 128, 16)
        for ci in range(nchunk):
            blockw = w[ci].reshape(-1, 16).T
            wrapped[c, :, 128 * ci:128 * (ci + 1)] = np.tile(blockw, (8, 1))
    return bpt, nblk, nchunk, wrapped, S_all, D_all


def kernel(**inputs):
    global LAST_RES
    _install_ntff_shim()
    SKIP_PROT = bool(int(os.environ.get("DTA_SKIP_PROT", "0")))
    SKIP_GCN = bool(int(os.environ.get("DTA_SKIP_GCN", "0")))
    F32R = bool(int(os.environ.get("DTA_F32R", "1")))
    PREP = bool(int(os.environ.get("DTA_PREP", "0")))
    import concourse.bacc as bacc
    import concourse.tile as tile
    from concourse import hw_specs
    # Tile's static schedule uses this cost model; the default badly
    # underestimates SWDGE gather descriptor generation (~8 ns/desc measured),
    # which starves the PE stream of filler work during gather windows.
    hw_specs.TRN2Spec.SWDGE_NS_PER_DESCRIPTOR = 8.0
    from concourse import mybir
    from concourse.bass_utils import run_bass_kernel_spmd
    from concourse.library_config import mlp as mlp_lib

    g = lambda k: np.ascontiguousarray(np.asarray(inputs[k], F32))
    x = g("x")
    batch = np.asarray(inputs["batch"], np.int64)
    seq = np.asarray(inputs["protein_seq"], np.int64)
    W1 = g("W1")
    W2 = g("W2")
    W3 = g("W3")
    g1, bt1, g2, bt2, g3, bt3 = g("g1"), g("bt1"), g("g2"), g("bt2"), g("g3"), g("bt3")
    emb_w = g("emb")
    ck1, cb1, ck2, cb2, ck3, cb3 = g("ck1"), g("cb1"), g("ck2"), g("cb2"), g("ck3"), g("cb3")
    fw1, fb1, fw2, fb2, fw3, fb3 = g("fw1"), g("fb1"), g("fw2"), g("fb2"), g("fw3"), g("fb3")

    # ---------------- host tables ----------------
    bpt, nblk, nchunk, idx_w, S_host, D_host = _prep_edges(
        np.asarray(inputs["edge_index"]))

    cnt = np.bincount(batch, minlength=BATCH).astype(F32)
    cntinv = (1.0 / np.maximum(cnt, 1.0)).astype(F32)
    # pooling one-hots: P_all[c][n, (w*NT + t)*128 + j] = (batch[node] == 128w + j)
    P_host = np.zeros((NCORES, 128, 4 * NT * 128), F32)
    for c in range(NCORES):
        bl = batch[NSH * c:NSH * (c + 1)].reshape(NT, 128)
        for t in range(NT):
            w = bl[t] // 128
            j = bl[t] % 128
            P_host[c, np.arange(128), (w * NT + t) * 128 + j] = 1.0

    chvec = np.zeros((128, 6), F32)
    chvec[:64, 0], chvec[:64, 1] = g1, bt1
    chvec[:, 2], chvec[:, 3] = g2, bt2
    chvec[:, 4], chvec[:, 5] = g3, bt3

    pidx = np.zeros((128, 1), F32)
    pidx[:52, 0] = np.tile(np.arange(26, dtype=F32), 2)
    ident = np.eye(128, dtype=F32)
    xpad = np.zeros((N_NODES, 64), F32)
    xpad[:, :5] = x

    ids4 = seq.astype(F32).reshape(NCORES, BSH // 4, 4, SEQ_LEN).transpose(0, 2, 1, 3) \
        .reshape(NCORES, 4, (BSH // 4) * SEQ_LEN).copy()
    # 16-partition repack: ids16[c, si*4+q, j] = ids4[c, si, q*4000+j]
    ids16 = ids4.reshape(NCORES, 4, 4, 4000).reshape(NCORES, 16, 4000).copy()
    # broadcast selectors for si-pairs: variant (sp, q) -> rows (si2, v)
    sel16 = np.zeros((16, 8 * 52), F32)
    for sp in range(2):
        for q in range(4):
            for s2 in range(2):
                base = (sp * 4 + q) * 52 + s2 * 26
                sel16[(2 * sp + s2) * 4 + q, base:base + 26] = 1.0
    # conv1 with embedding folded in: M1[t, v, o] = sum_c ck1[o,c,t]*emb[v,c],
    # si-pair block-diag [(si2, v)=52, (si2, o)=64], taps concatenated.
    M1 = np.einsum('oct,vc->tvo', ck1, emb_w).astype(F32)
    ck1n = np.zeros((4, 52, 64), F32)
    ck1n[:, 0:26, 0:32] = M1
    ck1n[:, 26:52, 32:64] = M1
    ck1T = np.ascontiguousarray(ck1n.transpose(1, 0, 2).reshape(52, 256))
    # conv2 tap-pairs, rows (delta, si2, c), cols (si2, o)
    ck2n = np.zeros((3, 2, 2, 32, 128), F32)
    for tp in range(3):
        for d in range(2):
            for s2 in range(2):
                ck2n[tp, d, s2, :, s2 * 64:s2 * 64 + 64] = ck2[:, :, 2 * tp + d].T
    ck2q = np.ascontiguousarray(
        ck2n.reshape(3, 128, 128).transpose(1, 0, 2).reshape(128, 384))
    # conv3 tap-pairs, rows (delta, c), cols o
    ck3n = np.zeros((4, 2, 64, 96), F32)
    for tp in range(4):
        for d in range(2):
            ck3n[tp, d] = ck3[:, :, 2 * tp + d].T
    ck3d = np.ascontiguousarray(
        ck3n.reshape(4, 128, 96).transpose(1, 0, 2).reshape(128, 384))
    cb1_2 = np.tile(cb1, 2)[:, None]
    fw1a = np.ascontiguousarray(fw1[:128])
    fw1b = np.ascontiguousarray(fw1[128:224])
    fb1c = np.ascontiguousarray(fb1.reshape(4, 128).T)
    fw2p = np.ascontiguousarray(fw2.reshape(4, 128, 256).transpose(1, 0, 2).reshape(128, 1024))
    fb2c = np.ascontiguousarray(fb2.reshape(2, 128).T)
    fw3p = np.ascontiguousarray(fw3.reshape(2, 128).T)

    # ---------------- device program ----------------
    dt = mybir.dt
    AF = mybir.ActivationFunctionType
    OP = mybir.AluOpType
    rg = [list(range(NCORES))]

    fr = dt.float32r if F32R else dt.float32

    nc = bacc.Bacc("TRN2", target_bir_lowering=False, debug=False, num_devices=NCORES)

    def din(name, shape, dty=dt.float32):
        return nc.dram_tensor(name, list(shape), dty, kind="ExternalInput")

    xpad_d = din("xpad", [N_NODES, 64], fr)
    xloc_d = din("xloc", [NSH, 64], fr)
    D_d = din("Dm", [128, NT * 128], fr)
    W1_d, W2_d, W3_d = din("W1", [5, 64], fr), din("W2", [64, 128], fr), din("W3", [128, 128], fr)
    chv_d = din("chv", [128, 6])
    ident_d = din("ident", [128, 128])
    pidx_d = din("pidx", [128, 1])
    idx_d = din("idxg", [128, nchunk * 128], dt.int16)
    S_d = din("Sm", [128, nblk * 128], fr)
    P_d = din("Pm", [128, 4 * NT * 128], fr)
    cntinv_d = din("cntinv", [64, 1])
    ids_d = din("ids16", [16, 4000], fr)
    sel_d = din("sel16", [16, 8 * 52], fr)
    ck1_d, ck2_d, ck3_d = (din("ck1T", [52, 256], fr), din("ck2q", [128, 384], fr),
                           din("ck3d", [128, 384], fr))
    cb1_d, cb2_d, cb3_d = din("cb1", [64, 1]), din("cb2", [64, 1]), din("cb3", [96, 1])
    f1a_d, f1b_d = din("fw1a", [128, 512], fr), din("fw1b", [96, 512], fr)
    fb1_d = din("fb1c", [128, 4])
    f2_d, fb2_d = din("fw2p", [128, 1024], fr), din("fb2c", [128, 2])
    f3_d, fb3_d = din("fw3p", [128, 2], fr), din("fb3", [1, 1])
    out_d = nc.dram_tensor("out", [1, BSH], dt.float32, kind="ExternalOutput")

    with tile.TileContext(nc) as tc:
        with (
            tc.tile_pool(name="dram", bufs=1, space="DRAM") as dram,
            tc.tile_pool(name="cst", bufs=1) as cst,
            tc.tile_pool(name="big", bufs=1) as big,
            tc.tile_pool(name="wk", bufs=2) as wk,
            tc.tile_pool(name="wk1", bufs=1) as wk1,
            tc.tile_pool(name="wk3", bufs=1) as wk3,
            tc.tile_pool(name="gc", bufs=3) as gc,
            tc.tile_pool(name="sld", bufs=3) as sld,
            tc.tile_pool(name="gp", bufs=2, space="PSUM") as gp,
            tc.tile_pool(name="pp", bufs=4, space="PSUM") as pp,
        ):
            nc.gpsimd.load_library(mlp_lib)

            ag1_in = dram.tile([NSH, 64], fr)
            h1_full = dram.tile([N_NODES, 64], fr, addr_space="Shared")
            ag2_in = dram.tile([NSH, 128], fr)
            h2_full = dram.tile([N_NODES, 128], fr, addr_space="Shared")
            ar_ins = [dram.tile([128, 2], dt.float32, name=f"arin{i}") for i in range(3)]
            ar_outs = [dram.tile([128, 2], dt.float32, addr_space="Shared",
                                 name=f"arout{i}") for i in range(3)]
            arp_in = dram.tile([BATCH, 128], dt.float32)
            rsp_out = dram.tile([BSH, 128], dt.float32)

            def load(name, d, shape, dty=dt.float32):
                t = cst.tile(shape, dty, name=name)
                nc.sync.dma_start(t[:], d[:])
                return t

            ident_t = load("ident_t", ident_d, [128, 128])
            pidx_t = load("pidx_t", pidx_d, [128, 1])
            chv_t = load("chv_t", chv_d, [128, 6])
            W1_t = load("W1_t", W1_d, [5, 64], fr)
            W2_t = load("W2_t", W2_d, [64, 128], fr)
            W3_t = load("W3_t", W3_d, [128, 128], fr)
            idx_t = load("idx_t", idx_d, [128, nchunk * 128], dt.int16)
            D_t = load("D_t", D_d, [128, NT * 128], fr)
            xloc_t = cst.tile([128, NT * 64], fr, name="xloc_t")
            nc.sync.dma_start(
                xloc_t[:].rearrange("p (t j) -> p t j", j=64),
                xloc_d[:, :].rearrange("(t p) j -> p t j", p=128))
            cntinv_t = load("cntinv_t", cntinv_d, [64, 1])
            ids_t = load("ids_t", ids_d, [16, 4000], fr)
            sel_t = load("sel_t", sel_d, [16, 8 * 52], fr)
            ck1_t = load("ck1_t", ck1_d, [52, 256], fr)
            ck2_t = load("ck2_t", ck2_d, [128, 384], fr)
            ck3_t = load("ck3_t", ck3_d, [128, 384], fr)
            cb1_t = load("cb1_t", cb1_d, [64, 1])
            cb2_t = load("cb2_t", cb2_d, [64, 1])
            cb3_t = load("cb3_t", cb3_d, [96, 1])
            f1a_t = load("f1a_t", f1a_d, [128, 512], fr)
            f1b_t = load("f1b_t", f1b_d, [96, 512], fr)
            fb1_t = load("fb1_t", fb1_d, [128, 4])
            f2_t = load("f2_t", f2_d, [128, 1024], fr)
            fb2_t = load("fb2_t", fb2_d, [128, 2])
            f3_t = load("f3_t", f3_d, [128, 2], fr)
            fb3_t = load("fb3_t", fb3_d, [1, 1])

            z_sb = big.tile([128, NSH], dt.float32)
            h3T = big.tile([128, NSH], dt.float32)
            hnat = big.tile([128, NSH], fr)
            protT = big.tile([96, BSH], fr)
            c1T = [big.tile([128, BSH], fr, name=f"c1T_{j}") for j in range(4)]
            c2T = [big.tile([128, BSH], fr, name=f"c2T_{j}") for j in range(2)]

            # ---------------- protein group ----------------
            # conv1 has the embedding folded into its weights (K = taps x 26
            # one-hot rows, si-pair block-diag M=64); conv2/conv3 use tap-pair
            # im2col rows (delta in K) with column-offset rhs windows.
            def protein_group(gi):
                oh2 = []
                for sp in range(2):
                    o2 = wk1.tile([52, 1004], fr, tag=f"oh{sp}", name=f"oh{gi}_{sp}")
                    nc.vector.memset(o2[:, 0:1].bitcast(dt.float32), 0.0)
                    nc.vector.memset(o2[:, 1001:1004].bitcast(dt.float32), 0.0)
                    for lc in range(2):
                        l0 = 500 * lc
                        off = SEQ_LEN * gi + l0
                        var = sp * 4 + off // 4000
                        col = off % 4000
                        pb = pp.tile([52, 500], dt.float32, space="PSUM", tag="pp",
                                     name=f"pb{gi}_{sp}_{lc}")
                        nc.tensor.matmul(
                            pb[:], lhsT=sel_t[:, 52 * var:52 * (var + 1)],
                            rhs=ids_t[:, col:col + 500], start=True, stop=True)
                        nc.vector.tensor_scalar(o2[:, 1 + l0:1 + l0 + 500], pb[:],
                                                pidx_t[:52, :], None, OP.is_equal)
                    oh2.append(o2)
                J2 = []
                for sp in range(2):
                    j2 = wk.tile([128, 1002], fr, tag=f"J2{sp}", name=f"J2{gi}_{sp}")
                    nc.vector.memset(j2[:, 0:2].bitcast(dt.float32), 0.0)
                    nc.vector.memset(j2[:, 1000:1002].bitcast(dt.float32), 0.0)
                    for (l0, Lm, Lc) in ((0, 512, 512), (512, 488, 487)):
                        c1p = pp.tile([64, 512], dt.float32, space="PSUM", tag="pp",
                                      name=f"c1p{gi}_{sp}_{l0}")
                        for tap in range(4):
                            nc.tensor.matmul(
                                c1p[:, :Lm], lhsT=ck1_t[:, 64 * tap:64 * (tap + 1)],
                                rhs=oh2[sp][:, l0 + tap:l0 + tap + Lm],
                                start=(tap == 0), stop=(tap == 3))
                        nc.scalar.activation(j2[0:64, l0 + 2:l0 + 2 + Lc],
                                             c1p[:, :Lc], AF.Relu, bias=cb1_t[:])
                        nc.scalar.activation(j2[64:128, l0 + 1:l0 + 1 + Lc],
                                             c1p[:, :Lc], AF.Relu, bias=cb1_t[:])
                    J2.append(j2)
                J3 = []
                for si in range(4):
                    j3 = wk1.tile([128, 1004], fr, tag=f"J3{si}", name=f"J3{gi}_{si}")
                    nc.vector.memset(j3[:, 0:3].bitcast(dt.float32), 0.0)
                    nc.vector.memset(j3[:, 1000:1004].bitcast(dt.float32), 0.0)
                    J3.append(j3)
                for sp in range(2):
                    for (l0, Lc) in ((0, 512), (512, 486)):
                        c2p = pp.tile([128, 512], dt.float32, space="PSUM", tag="pp",
                                      name=f"c2p{gi}_{sp}_{l0}")
                        for tp in range(3):
                            nc.tensor.matmul(
                                c2p[:, :Lc], lhsT=ck2_t[:, 128 * tp:128 * (tp + 1)],
                                rhs=J2[sp][:, l0 + 2 * tp:l0 + 2 * tp + Lc],
                                start=(tp == 0), stop=(tp == 2))
                        for si2 in range(2):
                            j3 = J3[2 * sp + si2]
                            nc.scalar.activation(
                                j3[0:64, l0 + 3:l0 + 3 + Lc],
                                c2p[64 * si2:64 * (si2 + 1), :Lc], AF.Relu,
                                bias=cb2_t[:])
                            nc.scalar.activation(
                                j3[64:128, l0 + 2:l0 + 2 + Lc],
                                c2p[64 * si2:64 * (si2 + 1), :Lc], AF.Relu,
                                bias=cb2_t[:])
                for si in range(4):
                    mx = wk.tile([96, 1], dt.float32, tag="mx0", name=f"mx{gi}_{si}")
                    tmp = wk.tile([96, 1], dt.float32, tag="mx1", name=f"tm{gi}_{si}")
                    for (l0, Lm, Lc) in ((0, 512, 512), (512, 486, 485)):
                        c3p = pp.tile([96, 512], dt.float32, space="PSUM", tag="pp",
                                      name=f"c3p{gi}_{si}_{l0}")
                        for tp in range(4):
                            nc.tensor.matmul(
                                c3p[:, :Lm], lhsT=ck3_t[:, 96 * tp:96 * (tp + 1)],
                                rhs=J3[si][:, l0 + 2 * tp:l0 + 2 * tp + Lm],
                                start=(tp == 0), stop=(tp == 3))
                        dst = mx if l0 == 0 else tmp
                        nc.vector.tensor_reduce(dst[:], c3p[:, :Lc],
                                                axis=mybir.AxisListType.X, op=OP.max)
                        if l0 != 0:
                            nc.vector.tensor_tensor(mx[:], mx[:], tmp[:], OP.max)
                    s_idx = 4 * gi + si
                    nc.scalar.activation(protT[:, s_idx:s_idx + 1], mx[:],
                                         AF.Relu, bias=cb3_t[:])

            pending = [] if SKIP_PROT else list(range(16))
            slot = [0]

            def filler(period=8):
                slot[0] += 1
                if pending and slot[0] % period == 0:
                    protein_group(pending.pop(0))

            # ---------------- GCN layer ----------------
            gsem = nc.alloc_semaphore("gsem")

            def gcn_layer(L, fg, fin, fout, src_dram, Wt, loc, lw):
                """fg: gathered row width; fin: contraction width of Wt.
                loc/lw: SBUF tile + per-tile stride holding the LOCAL input rows
                (natural layout) for the self-loop diag term."""
                zs = cst.tile([128, NT], dt.float32, name=f"zs{L}")
                zq = cst.tile([128, NT], dt.float32, name=f"zq{L}")
                sq_scr = wk.tile([128, 128], dt.float32, tag="sqs", name=f"sqs{L}")
                dsem = nc.alloc_semaphore(f"dsem{L}")
                chs = {}
                # The trigger's deferred RAW edge on src_dram is lost when the
                # writer is a collective; thread it through a probe tile: the
                # probe DMA waits on the AllGather, and the first trigger gets
                # a WAW edge on the probe via signals_writable.
                probe = None
                if L > 0:
                    probe = cst.tile([1, 64], fr, name=f"probe{L}")
                    nc.sync.dma_start(probe[:], src_dram[0:1, 0:64])
                sig = {"first": True}

                def emit_prep(ci, trigger=True):
                    g = gc.tile([128, 16, fg], fr, tag="gch", name=f"g{L}_{ci}")
                    chs[ci] = g
                    if PREP:
                        nc.gpsimd.dma_gather(
                            g[:], src_dram[:],
                            idx_t[:, 128 * ci:128 * (ci + 1)], 2048, 2048, fg,
                            single_packet=False, prepare_only=True, sem=dsem)
                        if not trigger:
                            return
                        if sig["first"] and probe is not None:
                            # Pool-engine read of the probe: gates this and all
                            # later triggers (Pool program order) on the
                            # AllGather that produced src_dram.
                            pdum = cst.tile([1, 64], fr, name=f"pdum{L}")
                            nc.gpsimd.tensor_copy(pdum[:], probe[:])
                        sig["first"] = False
                        nc.gpsimd.trigger_dma(count=None)
                    else:
                        nc.gpsimd.dma_gather(
                            g[:], src_dram[:],
                            idx_t[:, 128 * ci:128 * (ci + 1)], 2048, 2048, fg,
                            single_packet=False)

                PRE = 2
                for ci in range(min(PRE, nchunk)):
                    emit_prep(ci, trigger=(ci == min(PRE, nchunk) - 1))
                Gt = None
                for t in range(NT):
                    St = sld.tile([128, bpt * 128], fr, tag="Sld",
                                  name=f"S{L}_{t}")
                    nc.sync.dma_start(St[:], S_d[:, t * bpt * 128:(t + 1) * bpt * 128])
                    aggT = gp.tile([128, 128], dt.float32, space="PSUM", tag="aggp",
                                   name=f"agg{L}_{t}")
                    nc.tensor.matmul(aggT[:fin, :],
                                     lhsT=loc[:, lw * t:lw * t + fin],
                                     rhs=D_t[:, 128 * t:128 * (t + 1)],
                                     start=True, stop=False)
                    for k in range(bpt):
                        b = t * bpt + k
                        ci, bb = divmod(b, 16)
                        if bb == 0:
                            Gt = chs.pop(ci)
                            if ci + PRE < nchunk:
                                emit_prep(ci + PRE)
                        nc.tensor.matmul(aggT[:fin, :], lhsT=Gt[:, bb, :fin],
                                         rhs=St[:, 128 * k:128 * (k + 1)],
                                         start=False, stop=(k == bpt - 1))
                    aggS = wk.tile([fin, 128], fr, tag="aggS",
                                   name=f"aggS{L}_{t}")
                    nc.vector.tensor_copy(aggS[:], aggT[:fin, :])
                    zT = gp.tile([128, 128], dt.float32, space="PSUM", tag="zp",
                                 name=f"z{L}_{t}")
                    nc.tensor.matmul(zT[:fout, :], lhsT=Wt[:fin, :fout], rhs=aggS[:],
                                     start=True, stop=True)
                    nc.scalar.activation(z_sb[:fout, 128 * t:128 * (t + 1)], zT[:fout, :],
                                         AF.Copy, accum_out=zs[:fout, t:t + 1])
                    nc.scalar.activation(sq_scr[:fout, :], zT[:fout, :], AF.Square,
                                         accum_out=zq[:fout, t:t + 1])
                    filler(10)
                ssum = wk.tile([128, 2], dt.float32, tag="ssum", name=f"ssum{L}")
                nc.vector.memset(ssum[:], 0.0)
                nc.vector.tensor_reduce(ssum[:fout, 0:1], zs[:fout, :],
                                        axis=mybir.AxisListType.X, op=OP.add)
                nc.vector.tensor_reduce(ssum[:fout, 1:2], zq[:fout, :],
                                        axis=mybir.AxisListType.X, op=OP.add)
                nc.sync.dma_start(ar_ins[L][:], ssum[:])
                nc.gpsimd.collective_compute(
                    "AllReduce", OP.add, replica_groups=rg,
                    ins=[ar_ins[L].opt()], outs=[ar_outs[L].opt()])
                stg = wk.tile([128, 2], dt.float32, tag="stg", name=f"stg{L}")
                nc.sync.dma_start(stg[:], ar_outs[L][:])
                vg = chv_t[:fout, 2 * L:2 * L + 1]
                vbt = chv_t[:fout, 2 * L + 1:2 * L + 2]
                mean = wk.tile([128, 1], dt.float32, tag="bnv0", name=f"mean{L}")
                ex2 = wk.tile([128, 1], dt.float32, tag="bnv1", name=f"ex2{L}")
                var = wk.tile([128, 1], dt.float32, tag="bnv2", name=f"var{L}")
                sd = wk.tile([128, 1], dt.float32, tag="bnv3", name=f"sd{L}")
                s_ch = wk.tile([128, 1], dt.float32, tag="bnv4", name=f"sch{L}")
                b_ch = wk.tile([128, 1], dt.float32, tag="bnv5", name=f"bch{L}")
                t1 = wk.tile([128, 1], dt.float32, tag="bnv6", name=f"t1{L}")
                nc.vector.tensor_scalar(mean[:fout], stg[:fout, 0:1], 1.0 / N_NODES,
                                        None, OP.mult)
                nc.vector.tensor_scalar(ex2[:fout], stg[:fout, 1:2], 1.0 / N_NODES,
                                        None, OP.mult)
                nc.vector.tensor_tensor(var[:fout], mean[:fout], mean[:fout], OP.mult)
                nc.vector.tensor_tensor(var[:fout], ex2[:fout], var[:fout], OP.subtract)
                nc.vector.tensor_scalar(var[:fout], var[:fout], 1e-5, None, OP.add)
                nc.scalar.activation(sd[:fout], var[:fout], AF.Sqrt)
                nc.vector.reciprocal(s_ch[:fout], sd[:fout])
                nc.vector.tensor_tensor(s_ch[:fout], s_ch[:fout], vg, OP.mult)
                nc.vector.tensor_tensor(t1[:fout], mean[:fout], s_ch[:fout], OP.mult)
                nc.vector.tensor_tensor(b_ch[:fout], vbt, t1[:fout], OP.subtract)
                return s_ch, b_ch

            def apply_bn(L, fout, s_ch, b_ch, to_h3T):
                for t in range(NT):
                    if to_h3T:
                        nc.scalar.activation(
                            h3T[:fout, 128 * t:128 * (t + 1)],
                            z_sb[:fout, 128 * t:128 * (t + 1)],
                            AF.Relu, bias=b_ch[:fout], scale=s_ch[:fout])
                    else:
                        hT = wk.tile([128, 128], dt.float32, tag="hT", name=f"hT{L}_{t}")
                        nc.scalar.activation(
                            hT[:fout, :], z_sb[:fout, 128 * t:128 * (t + 1)],
                            AF.Relu, bias=b_ch[:fout], scale=s_ch[:fout])
                        tp = gp.tile([128, 128], dt.float32, space="PSUM", tag="zp",
                                     name=f"tp{L}_{t}")
                        nc.tensor.transpose(tp[:, :fout], hT[:fout, :],
                                            ident_t[:fout, :fout])
                        nc.vector.tensor_copy(hnat[:, fout * t:fout * (t + 1)],
                                              tp[:, :fout])

            def _emit_regressor(drugT):
                for jc in range(4):
                    f1ps = pp.tile([128, 64], dt.float32, space="PSUM", tag="pp",
                                   name=f"f1ps{jc}")
                    nc.tensor.matmul(f1ps[:], lhsT=f1a_t[:, 128 * jc:128 * (jc + 1)],
                                     rhs=drugT[:], start=True, stop=False)
                    nc.tensor.matmul(f1ps[:], lhsT=f1b_t[:, 128 * jc:128 * (jc + 1)],
                                     rhs=protT[:], start=False, stop=True)
                    nc.scalar.activation(c1T[jc][:, :], f1ps[:], AF.Relu,
                                         bias=fb1_t[:, jc:jc + 1])
                for jc in range(2):
                    f2ps = pp.tile([128, 64], dt.float32, space="PSUM", tag="pp",
                                   name=f"f2ps{jc}")
                    for ic in range(4):
                        nc.tensor.matmul(
                            f2ps[:],
                            lhsT=f2_t[:, 256 * ic + 128 * jc:256 * ic + 128 * jc + 128],
                            rhs=c1T[ic][:, :], start=(ic == 0), stop=(ic == 3))
                    nc.scalar.activation(c2T[jc][:, :], f2ps[:], AF.Relu,
                                         bias=fb2_t[:, jc:jc + 1])
                f3ps = pp.tile([1, 64], dt.float32, space="PSUM", tag="pp", name="f3ps0")
                for ic in range(2):
                    nc.tensor.matmul(f3ps[:], lhsT=f3_t[:, ic:ic + 1],
                                     rhs=c2T[ic][:, :],
                                     start=(ic == 0), stop=(ic == 1))
                outs = wk.tile([1, 64], dt.float32, tag="outs", name="outs0")
                nc.vector.tensor_scalar(outs[:], f3ps[:], fb3_t[:1, 0:1], None, OP.add)
                nc.sync.dma_start(out_d[:], outs[:])

            # ================= emission =================
            if SKIP_GCN:
                for gi in list(pending):
                    protein_group(gi)
                pending.clear()
                drugT0 = wk.tile([128, 64], fr, tag="drugT", name="drugT0")
                nc.vector.memset(drugT0[:].bitcast(dt.float32), 0.0)
                _emit_regressor(drugT0)
            else:
                if pending:
                    protein_group(pending.pop(0))
                s1, bb1 = gcn_layer(0, 64, 5, 64, xpad_d, W1_t, xloc_t, 64)
                apply_bn(0, 64, s1, bb1, False)
                view1 = ag1_in[:, :].rearrange("(t p) j -> p t j", p=128)
                nc.sync.dma_start(view1,
                                  hnat[:, :NT * 64].rearrange("p (t j) -> p t j", j=64))
                nc.gpsimd.collective_compute("AllGather", OP.bypass, replica_groups=rg,
                                             ins=[ag1_in.opt()], outs=[h1_full.opt()])
                filler(1)
                filler(1)
                filler(1)

                s2c, bb2 = gcn_layer(1, 64, 64, 128, h1_full, W2_t, hnat, 64)
                apply_bn(1, 128, s2c, bb2, False)
                view2 = ag2_in[:, :].rearrange("(t p) j -> p t j", p=128)
                nc.sync.dma_start(view2,
                                  hnat[:, :NT * 128].rearrange("p (t j) -> p t j", j=128))
                nc.gpsimd.collective_compute("AllGather", OP.bypass, replica_groups=rg,
                                             ins=[ag2_in.opt()], outs=[h2_full.opt()])
                filler(1)
                filler(1)
                filler(1)

                s3c, bb3 = gcn_layer(2, 128, 128, 128, h2_full, W3_t, hnat, 128)
                apply_bn(2, 128, s3c, bb3, True)

                for t in range(NT):
                    tpp = gp.tile([128, 128], dt.float32, space="PSUM", tag="zp",
                                  name=f"tpp_{t}")
                    nc.tensor.transpose(tpp[:], h3T[:, 128 * t:128 * (t + 1)], ident_t[:])
                    nc.vector.tensor_copy(hnat[:, 128 * t:128 * (t + 1)], tpp[:])
                    filler(4)
                for w in range(4):
                    poolw = gp.tile([128, 128], dt.float32, space="PSUM", tag="aggp",
                                    name=f"poolps{w}")
                    Pw = wk3.tile([128, NT * 128], fr, tag="p1h", name=f"p1_{w}")
                    nc.sync.dma_start(
                        Pw[:], P_d[:, w * NT * 128:(w + 1) * NT * 128])
                    for t in range(NT):
                        nc.tensor.matmul(
                            poolw[:], lhsT=Pw[:, 128 * t:128 * (t + 1)],
                            rhs=hnat[:, 128 * t:128 * (t + 1)],
                            start=(t == 0), stop=(t == NT - 1))
                    parts = wk.tile([128, 128], dt.float32, tag="parts", name=f"parts{w}")
                    nc.vector.tensor_copy(parts[:], poolw[:])
                    nc.sync.dma_start(arp_in[128 * w:128 * (w + 1), :], parts[:])
                    filler(2)
                nc.gpsimd.collective_compute("ReduceScatter", OP.add, replica_groups=rg,
                                             ins=[arp_in.opt()], outs=[rsp_out.opt()])
                drugsum = wk.tile([64, 128], dt.float32, tag="drugsum", name="drugsum0")
                nc.sync.dma_start(drugsum[:], rsp_out[:])
                drug = wk.tile([64, 128], dt.float32, tag="drug", name="drug0")
                nc.vector.tensor_scalar(drug[:], drugsum[:], cntinv_t[:], None, OP.mult)
                tpd = gp.tile([128, 128], dt.float32, space="PSUM", tag="zp", name="tpd0")
                nc.tensor.transpose(tpd[:, :64], drug[:], ident_t[:64, :64])
                drugT = wk.tile([128, 64], fr, tag="drugT", name="drugT0")
                nc.vector.tensor_copy(drugT[:], tpd[:, :64])

                while pending:
                    protein_group(pending.pop(0))
                _emit_regressor(drugT)

    nc.compile()

    if os.environ.get("DTA_DUMP_TRIG"):
        import json as _json
        recs = []
        for f in nc.m.functions:
            for bb in f.blocks:
                for ins in bb.instructions:
                    nm = type(ins).__name__
                    si = getattr(ins, "sync_info", None)
                    s = str(si)
                    if ("Trigger" in nm or "Gather" in nm or "Collective" in nm
                            or "dsem" in s):
                        eng = getattr(ins, "engine", "?")
                        recs.append((nm, ins.name, str(eng), s[:400]))
        with open("/tmp/trig_dump.txt", "w") as fh:
            for r in recs:
                fh.write(repr(r) + "\n")
        print(f"dumped {len(recs)} records to /tmp/trig_dump.txt")
        raise SystemExit(0)

    in_maps = []
    for c in range(NCORES):
        in_maps.append({
            "xpad": xpad, "xloc": xpad[NSH * c:NSH * (c + 1)],
            "W1": W1, "W2": W2, "W3": W3, "chv": chvec,
            "ident": ident, "pidx": pidx,
            "idxg": idx_w[c], "Sm": S_host[c], "Pm": P_host[c], "Dm": D_host[c],
            "cntinv": cntinv[64 * c:64 * (c + 1)][:, None],
            "ids16": ids16[c], "sel16": sel16,
            "ck1T": ck1T, "ck2q": ck2q, "ck3d": ck3d,
            "cb1": cb1_2, "cb2": cb2[:, None], "cb3": cb3[:, None],
            "fw1a": fw1a, "fw1b": fw1b, "fb1c": fb1c,
            "fw2p": fw2p, "fb2c": fb2c, "fw3p": fw3p,
            "fb3": np.array([[fb3[0]]], F32),
        })

    res = run_bass_kernel_spmd(nc, in_maps, core_ids=list(range(NCORES)))
    LAST_RES = res
    out = np.concatenate([res.results[c]["out"][0] for c in range(NCORES)])
    return out.astype(F32)

